# revision 39
# baseline (speedup 1.0000x reference)
"""GAT layer (DGL GATConv + ELU + residual) as a Bass/Tile kernel on 8 TRN2 NeuronCores.

Strategy (edge parallelism, dst-sharded; v2 — fp8 gather table, fused el,
on-chip er expansion):
  - Sort edges by (dst-window, src-region, src) on host; shard contiguous
    dst-node ranges across the 8 cores (6272 nodes/core = 49 windows of 128).
    Each core owns all incoming edges of its node range, so softmax +
    scatter-add are core-local and no collective is needed.
  - Phase A (replicated on every core): one pass of h @ [W | W@Al] produces
    gather-table rows [feat fp8e4m3 (256B) | el fp8 (4B) | pad] in 512B slots,
    written to DRAM ([50176, 512B]).  el = <feat, attn_l> rides along as 4
    extra GEMM columns (W@Al precomputed on host), so no per-edge el compute.
  - Phase A2: er_own = hTo^T @ (W@Ar) for the core's own 6272 nodes, kept
    SBUF-resident ([128, 49*4] bf16).
  - Phase B per 128-node window: one dma_gather per src-region (3 regions of
    <32768 rows each, so int16 indices reach them; slot pads gather row 0 of
    the region and are masked by the one-hots).  Dst-local one-hots are built on DVE
    in BOTH orientations (oh1[edge, node] batched dual-broadcast is_equal,
    ohT[node, edge] from a host-uploaded replicated dst-local int8 row).
    ohT expands er to edges via tiny PE matmuls; oh1 scatter-adds the
    exp-weighted bf16 messages + softmax denominators into a PSUM
    accumulator ([128, 260] f32).  Finalize: /denom, +bias, ELU, +h residual,
    bf16 output (upcast on host).
"""

import sys

for p in ("/opt/trn_rl_repo",):
    if p not in sys.path:
        sys.path.insert(0, p)

import numpy as np

import concourse.bass as bass
import concourse.bacc as bacc
import concourse.mybir as mybir
import concourse.tile as tile
from concourse.bass_utils import run_bass_kernel_spmd

F32 = mybir.dt.float32
BF16 = mybir.dt.bfloat16
FP8 = mybir.dt.float8e4
I8 = mybir.dt.int8
I16 = mybir.dt.int16
AF = mybir.ActivationFunctionType
OP = mybir.AluOpType

P = 128          # partitions / window size
ROWB = 512       # gather-table row bytes (fp8 elems)
ELO = 256        # byte offset of el fp8x4 within a row
ROWP = 260       # payload bytes per row (256 feat + 4 el, all fp8)
RSPLIT = 16768   # src-region width (< 32768 so int16 indices reach all rows)
GMAXT = 8        # max 128-idx tiles per dma_gather (desc-ring capacity 1024)

NP_BF16 = mybir.dt.np(BF16)
NP_FP8 = mybir.dt.np(FP8)


class Cfg:
    def __init__(self, N=50000, E=800000, H=4, D=64, ncores=8, nwin=49,
                 neg_slope=0.2):
        self.N, self.E, self.H, self.D = N, E, H, D
        self.F = H * D
        self.ncores = ncores
        self.nwin = nwin                  # windows (128 nodes) per core
        self.npc = nwin * P               # nodes per core
        self.npad = self.npc * ncores     # padded total nodes
        assert self.npad >= N
        self.neg_slope = neg_slope
        self.nreg = 3


def preprocess(cfg, src, dst):
    """Sort edges by (dst-window, src-region, src); build slot layouts with
    idx=-1 pads (skipped by dma_gather)."""
    s0 = np.asarray(src).astype(np.int64)
    d0 = np.asarray(dst).astype(np.int64)
    gwin = d0 // P
    region = np.minimum(s0 // RSPLIT, cfg.nreg - 1)
    order = np.lexsort((s0, region, gwin))
    s, d, region = s0[order], d0[order], region[order]
    dloc = (d % P).astype(np.int64)
    ngw = cfg.ncores * cfg.nwin
    nreg = cfg.nreg
    cnt_wr = np.zeros((ngw, nreg), np.int64)
    np.add.at(cnt_wr, (gwin[order], region), 1)
    starts = np.concatenate([[0], np.cumsum(cnt_wr.reshape(-1))])
    t_regs = tuple(int(np.ceil(cnt_wr[:, r].max() / P)) for r in range(nreg))
    tpw = sum(t_regs)
    slots = tpw * P
    tbase = np.concatenate([[0], np.cumsum(t_regs)])  # tile offset per region
    idxs = np.zeros((ngw, slots), np.int16)          # pad -> row 0 of region
    dstl = np.full((ngw, slots), 200.0, np.float32)  # pad -> no one-hot match
    dstl8 = np.full((ngw, slots), -1, np.int8)       # pad -> no match vs iota
    for g in range(ngw):
        for r in range(nreg):
            a = starts[g * nreg + r]
            b = starts[g * nreg + r + 1]
            n = b - a
            if n == 0:
                continue
            sb = int(tbase[r]) * P
            idxs[g, sb:sb + n] = s[a:b] - r * RSPLIT
            dstl[g, sb:sb + n] = dloc[a:b]
            dstl8[g, sb:sb + n] = dloc[a:b]
    # dma_gather consumes idx j from [j%16, j//16] (16-row block replicated
    # across the 8 groups of 16 partitions)
    def _dev_layout(a):
        cols = a.reshape(cfg.ncores, cfg.nwin * slots)
        dev = np.zeros((cfg.ncores, P, cfg.nwin * slots // 16), np.int16)
        for c in range(cfg.ncores):
            blk = cols[c].reshape(-1, 16).T          # [16, cols]
            dev[c] = np.tile(blk, (8, 1))
        return dev
    idx_dev = _dev_layout(idxs)
    # dstl (one-hot scalar operand): gather writes slot j -> [j%128, j//128];
    # tile t of window w is dstl_col[:, w*tpw + t]
    dc = dstl.reshape(cfg.ncores, cfg.nwin, tpw, P).transpose(0, 3, 1, 2)
    dstl_col = np.ascontiguousarray(dc.reshape(cfg.ncores, P, cfg.nwin * tpw))
    # dstl_rep (transposed one-hot operand): slot order along the free dim,
    # replicated across all 128 partitions
    dr = dstl8.reshape(cfg.ncores, cfg.nwin * slots)
    dstl_rep = np.ascontiguousarray(
        np.broadcast_to(dr[:, None, :], (cfg.ncores, P, cfg.nwin * slots)))
    return idx_dev, dstl_col, dstl_rep, t_regs


def build(cfg, t_regs, repeat=1, ablate=None, sim_safe=False,
          bias_zero=False):
    """Build the SPMD Bass program. repeat>1 re-emits the computation for
    device-time measurement ((t_k - t_1)/(k-1) cancels dispatch overhead).
    ablate: None | "aonly" | "nogather" | "noscatter" (timing diagnostics).
    sim_safe=True pins all gathers to SWDGE queue 0 (CoreSim cannot model
    multi-queue completion sems; HW counting sems make it safe)."""
    N, F, H, D = cfg.npad, cfg.F, cfg.H, cfg.D
    nwin, npc = cfg.nwin, cfg.npc
    nreg = cfg.nreg
    tpw = sum(t_regs)
    tbase = np.concatenate([[0], np.cumsum(t_regs)]).astype(int)
    KB = F // P            # contraction blocks (2)
    NBC = 1024             # phase-A node chunk
    assert N % NBC == 0
    nchunks = N // NBC
    tpn = NBC // P         # node tiles per chunk (8)
    FE = F + H             # 260 useful GEMM columns (feat + el)
    KW = 7                 # windows per batched finalize group
    assert nwin % KW == 0

    nc = bacc.Bacc("TRN2", target_bir_lowering=False, debug=False,
                   num_devices=cfg.ncores, num_swdge_queues=3)

    hT = nc.dram_tensor("hT", [F, N], BF16, kind="ExternalInput")
    hTo = nc.dram_tensor("hTo", [F, npc], BF16, kind="ExternalInput")
    ho = nc.dram_tensor("ho", [npc, F], BF16, kind="ExternalInput")
    W2 = nc.dram_tensor("W2", [F, FE], BF16, kind="ExternalInput")
    War = nc.dram_tensor("War", [F, H], BF16, kind="ExternalInput")
    brep = nc.dram_tensor("brep", [P, F], F32, kind="ExternalInput")
    idx_d = nc.dram_tensor("idx16", [P, nwin * tpw * P // 16], I16,
                           kind="ExternalInput")
    dstlc_d = nc.dram_tensor("dstlc", [P, nwin * tpw], F32,
                             kind="ExternalInput")
    dstlr_d = nc.dram_tensor("dstlr", [P, nwin * tpw * P], I8,
                             kind="ExternalInput")
    iotaf_d = nc.dram_tensor("iotaf", [P, P], BF16, kind="ExternalInput")
    iotap_d = nc.dram_tensor("iotap", [P, 1], F32, kind="ExternalInput")
    out_d = nc.dram_tensor("out", [npc, F], BF16, kind="ExternalOutput")

    with tile.TileContext(nc) as tc:
        with (
            tc.tile_pool(name="const", bufs=1) as cp,
            tc.tile_pool(name="dram", bufs=1, space="DRAM") as dp,
            tc.tile_pool(name="pa", bufs=3) as pa,
            tc.tile_pool(name="paps", bufs=2, space="PSUM") as paps,
            tc.tile_pool(name="a2ps", bufs=1, space="PSUM") as a2ps,
            tc.tile_pool(name="pg", bufs=4) as pg,
            tc.tile_pool(name="pb", bufs=3) as pb,
            tc.tile_pool(name="pbps", bufs=2, space="PSUM") as pbps,
            tc.tile_pool(name="erps", bufs=2, space="PSUM") as erps,
            tc.tile_pool(name="fin", bufs=2) as fin,
        ):
            # ---------------- constants ----------------
            w_sb = cp.tile([P, KB * FE], BF16)
            for k in range(KB):
                nc.sync.dma_start(w_sb[:, k * FE:(k + 1) * FE],
                                  W2[k * P:(k + 1) * P, :])
            war_sb = cp.tile([P, KB * H], BF16)
            for k in range(KB):
                nc.sync.dma_start(war_sb[:, k * H:(k + 1) * H],
                                  War[k * P:(k + 1) * P, :])
            brep2_sb = cp.tile([P, FE], F32)
            nc.sync.dma_start(brep2_sb[:, 0:F], brep[:])
            nc.vector.memset(brep2_sb[:, F:FE], 0.0)
            idx_sb = cp.tile([P, nwin * tpw * P // 16], I16)
            nc.sync.dma_start(idx_sb[:], idx_d[:])
            dstlc_sb = cp.tile([P, nwin * tpw], F32)
            nc.sync.dma_start(dstlc_sb[:], dstlc_d[:])
            iota_f = cp.tile([P, P], BF16)
            nc.sync.dma_start(iota_f[:], iotaf_d[:])
            iop_f = cp.tile([P, 1], F32)
            nc.sync.dma_start(iop_f[:], iotap_d[:])
            hto_sb = cp.tile([P, KB * npc], BF16)
            for k in range(KB):
                nc.sync.dma_start(hto_sb[:, k * npc:(k + 1) * npc],
                                  hTo[k * P:(k + 1) * P, :])
            er_sb = cp.tile([P, nwin * H], BF16)

            Tfeat = dp.tile([N, ROWB], FP8)
            gidx = [0]   # global gather counter: DMASW lane = gidx%8 is
                         # queue-locked, so queue must be a function of it

            def _emit_phases():
                # ---------------- phase A2: own-range er ----------------
                for w in range(nwin):
                    ps = a2ps.tile([P, H], F32, tag="a2")
                    for k in range(KB):
                        nc.tensor.matmul(
                            ps[:],
                            lhsT=hto_sb[:, k * npc + w * P:k * npc + (w + 1) * P],
                            rhs=war_sb[:, k * H:(k + 1) * H],
                            start=(k == 0), stop=(k == KB - 1))
                    nc.vector.tensor_copy(er_sb[:, w * H:(w + 1) * H], ps[:])

                # ---------------- phase A: gather table ----------------
                for b in range(nchunks):
                    hts = []
                    for k in range(KB):
                        ht = pa.tile([P, NBC], BF16, tag="ht")
                        nc.sync.dma_start(
                            ht[:], hT[k * P:(k + 1) * P, b * NBC:(b + 1) * NBC])
                        hts.append(ht)
                    fo = pa.tile([P, tpn * ROWP], FP8, tag="fo")
                    for i in range(tpn):
                        ps = paps.tile([P, FE], F32, tag="pa")
                        for k in range(KB):
                            nc.tensor.matmul(
                                ps[:], lhsT=hts[k][:, i * P:(i + 1) * P],
                                rhs=w_sb[:, k * FE:(k + 1) * FE],
                                start=(k == 0), stop=(k == KB - 1))
                        nc.vector.tensor_tensor(
                            out=fo[:, i * ROWP:(i + 1) * ROWP], in0=ps[:],
                            in1=brep2_sb[:], op=OP.add)
                    nc.sync.dma_start(
                        Tfeat[b * NBC:(b + 1) * NBC, 0:ROWP].rearrange(
                            "(i p) f -> p i f", p=P),
                        fo[:].rearrange("p (i f) -> p i f", f=ROWP))

                # ---------------- phase B: edges ----------------
                if ablate == "aonly":
                    for w in range(nwin):
                        how = fin.tile([P, F], BF16, tag="how")
                        nc.sync.dma_start(how[:], ho[w * P:(w + 1) * P, :])
                        nc.sync.dma_start(out_d[w * P:(w + 1) * P, :], how[:])
                    return
                for w in range(nwin):
                    base = w * tpw
                    icol = base * P // 16      # idx col base for this window
                    G = pg.tile([P, tpw * ROWB], FP8, tag="G")
                    if ablate == "nogather":
                        nc.vector.memset(G[:, 0:8], 0.0)
                    else:
                        for r in range(nreg):
                            for c0 in range(0, t_regs[r], GMAXT):
                                ct = min(GMAXT, t_regs[r] - c0)
                                rt0 = int(tbase[r]) + c0
                                nc.gpsimd.dma_gather(
                                    out_ap=G[:, rt0 * ROWB:(rt0 + ct) * ROWB]
                                        .rearrange("p (t f) -> p t f", f=ROWB),
                                    in_ap=Tfeat[r * RSPLIT:
                                                min((r + 1) * RSPLIT, N), :],
                                    idxs_ap=idx_sb[:, icol + rt0 * 8:
                                                   icol + (rt0 + ct) * 8],
                                    num_idxs=ct * P, num_idxs_reg=ct * P,
                                    elem_size=ROWB,
                                    queue_num=0 if sim_safe
                                    else (gidx[0] % 8) % 3)
                                gidx[0] += 1
                    g3 = G[:].rearrange("p (t f) -> p t f", f=ROWB)

                    # one-hots in both orientations
                    dr = pb.tile([P, tpw * P], I8, tag="dr")
                    nc.sync.dma_start(dr[:], dstlr_d[:, base * P:(base + tpw) * P])
                    oh1 = pb.tile([P, tpw * P], BF16, tag="oh1")
                    nc.vector.tensor_tensor(
                        out=oh1[:].rearrange("p (t j) -> p t j", j=P),
                        in0=iota_f[:].unsqueeze(1).to_broadcast([P, tpw, P]),
                        in1=dstlc_sb[:, base:base + tpw].unsqueeze(2)
                            .to_broadcast([P, tpw, P]),
                        op=OP.is_equal)
                    ohT = pb.tile([P, tpw * P], BF16, tag="ohT")
                    nc.vector.tensor_scalar(
                        out=ohT[:], in0=dr[:], scalar1=iop_f[:],
                        scalar2=None, op0=OP.is_equal)

                    # er[dst] per edge via transposed one-hot matmul
                    erp = erps.tile([P, tpw * H], F32, tag="erp")
                    for t in range(tpw):
                        nc.tensor.matmul(
                            erp[:, t * H:(t + 1) * H],
                            lhsT=ohT[:, t * P:(t + 1) * P],
                            rhs=er_sb[:, w * H:(w + 1) * H],
                            start=True, stop=True)

                    # scores -> leaky relu -> exp
                    sc = pb.tile([P, tpw * H], F32, tag="sc")
                    nc.vector.tensor_tensor(
                        out=sc[:].rearrange("p (t h) -> p t h", h=H),
                        in0=g3[:, :, ELO:ROWP],
                        in1=erp[:].rearrange("p (t h) -> p t h", h=H),
                        op=OP.add)
                    e1 = pb.tile([P, tpw * H], F32, tag="e1")
                    nc.scalar.activation(e1[:], sc[:], AF.Exp)
                    e2 = pb.tile([P, tpw * H], F32, tag="e2")
                    nc.scalar.activation(e2[:], sc[:], AF.Exp,
                                         scale=cfg.neg_slope)
                    ex = pb.tile([P, tpw * H], F32, tag="ex")
                    nc.vector.tensor_tensor(out=ex[:], in0=e1[:], in1=e2[:],
                                            op=OP.max)

                    # msg = feat_src * ex (broadcast over d), ex at col 256;
                    # ex is cast to bf16 once (msgx cols 256:260) and reused as
                    # the mult operand so numerator/denominator stay consistent
                    msgx = pb.tile([P, tpw * FE], BF16, tag="msgx")
                    mx3 = msgx[:].rearrange("p (t x) -> p t x", x=FE)
                    nc.vector.tensor_copy(
                        mx3[:, :, F:FE], ex[:].rearrange("p (t h) -> p t h", h=H))
                    nc.vector.tensor_tensor(
                        out=mx3[:, :, 0:F].rearrange(
                            "p t (h d) -> p t h d", d=D),
                        in0=g3[:, :, 0:F].rearrange(
                            "p t (h d) -> p t h d", d=D),
                        in1=mx3[:, :, F:FE].unsqueeze(3)
                            .to_broadcast([P, tpw, H, D]),
                        op=OP.mult)

                    # scatter-add into node accumulator
                    acc = pbps.tile([P, FE], F32, tag="acc")
                    if ablate == "noscatter":
                        nc.tensor.matmul(acc[:], lhsT=oh1[:, 0:P],
                                         rhs=msgx[:, 0:FE],
                                         start=True, stop=True)
                    else:
                        for t in range(tpw):
                            nc.tensor.matmul(
                                acc[:], lhsT=oh1[:, t * P:(t + 1) * P],
                                rhs=msgx[:, t * FE:(t + 1) * FE],
                                start=(t == 0), stop=(t == tpw - 1))

                    # finalize window
                    how = fin.tile([P, F], BF16, tag="how")
                    nc.sync.dma_start(how[:], ho[w * P:(w + 1) * P, :])
                    den = fin.tile([P, H], F32, tag="den")
                    nc.vector.tensor_scalar_max(den[:], acc[:, F:FE], 1e-30)
                    rden = fin.tile([P, H], F32, tag="rden")
                    nc.vector.reciprocal(rden[:], den[:])
                    rst = fin.tile([P, F], F32, tag="rst")
                    nc.vector.tensor_tensor(
                        out=rst[:].rearrange("p (h d) -> p h d", d=D),
                        in0=acc[:, 0:F].rearrange("p (h d) -> p h d", d=D),
                        in1=rden[:].unsqueeze(2).to_broadcast([P, H, D]),
                        op=OP.mult)
                    # ELU: max(x,0)-1 + exp(min(x,0)); then + h residual
                    emin = fin.tile([P, F], F32, tag="emin")
                    nc.vector.tensor_scalar_min(emin[:], rst[:], 0.0)
                    eexp = fin.tile([P, F], F32, tag="eexp")
                    nc.scalar.activation(eexp[:], emin[:], AF.Exp)
                    nc.vector.tensor_scalar(out=rst[:], in0=rst[:],
                                            scalar1=0.0, scalar2=-1.0,
                                            op0=OP.max, op1=OP.add)
                    ot = fin.tile([P, F], BF16, tag="ot")
                    nc.vector.tensor_tensor(out=ot[:], in0=rst[:],
                                            in1=eexp[:], op=OP.add)
                    nc.vector.tensor_tensor(out=ot[:], in0=ot[:], in1=how[:],
                                            op=OP.add)
                    nc.sync.dma_start(out_d[w * P:(w + 1) * P, :], ot[:])

            for _rep in range(repeat):
                _emit_phases()

    nc.compile()
    return nc


def make_in_maps(cfg, idx_dev, dstl_col, dstl_rep, h, W, attn_l, attn_r, bias):
    F, H, D = cfg.F, cfg.H, cfg.D
    h = np.asarray(h, np.float64)
    W64 = np.asarray(W, np.float64)
    Al = np.zeros((F, H))
    Ar = np.zeros((F, H))
    al = np.asarray(attn_l, np.float64)
    ar = np.asarray(attn_r, np.float64)
    for hh in range(H):
        Al[hh * D:(hh + 1) * D, hh] = al[hh]
        Ar[hh * D:(hh + 1) * D, hh] = ar[hh]
    W2 = np.concatenate([W64, W64 @ Al], axis=1).astype(NP_BF16)   # [F, 260]
    War = (W64 @ Ar).astype(NP_BF16)                               # [F, 4]

    h_pad = np.zeros((cfg.npad, F), np.float64)
    h_pad[:cfg.N] = h
    hT = np.ascontiguousarray(h_pad.T).astype(NP_BF16)
    brep = np.tile(np.asarray(bias, np.float32).reshape(1, F), (P, 1))
    iotaf = np.tile(np.arange(P, dtype=np.float32)[None, :],
                    (P, 1)).astype(NP_BF16)
    iotap = np.arange(P, dtype=np.float32)[:, None].copy()
    in_maps = []
    for c in range(cfg.ncores):
        lo, hi = c * cfg.npc, (c + 1) * cfg.npc
        in_maps.append({
            "hT": hT,
            "hTo": np.ascontiguousarray(hT[:, lo:hi]),
            "ho": h_pad[lo:hi].astype(NP_BF16),
            "W2": W2,
            "War": War,
            "brep": brep,
            "idx16": idx_dev[c],
            "dstlc": dstl_col[c],
            "dstlr": dstl_rep[c],
            "iotaf": iotaf,
            "iotap": iotap,
        })
    return in_maps


_CACHE = {}


def _run(cfg, inputs, **spmd_kwargs):
    h = np.asarray(inputs["h"], np.float32)
    W = np.asarray(inputs["W"], np.float32)
    attn_l = np.asarray(inputs["attn_l"], np.float32)
    attn_r = np.asarray(inputs["attn_r"], np.float32)
    bias = np.asarray(inputs["bias"], np.float32)
    src = np.asarray(inputs["src"])
    dst = np.asarray(inputs["dst"])

    idx_dev, dstl_col, dstl_rep, t_regs = preprocess(cfg, src, dst)
    bz = not np.asarray(inputs["bias"]).any()
    key = (cfg.N, cfg.E, cfg.ncores, cfg.nwin, t_regs, bz)
    if key not in _CACHE:
        _CACHE[key] = build(cfg, t_regs, bias_zero=bz)
    nc = _CACHE[key]
    in_maps = make_in_maps(cfg, idx_dev, dstl_col, dstl_rep, h, W,
                           attn_l, attn_r, bias)
    res = run_bass_kernel_spmd(nc, in_maps, list(range(cfg.ncores)),
                               **spmd_kwargs)
    outs = [res.results[c]["out"] for c in range(cfg.ncores)]
    full = np.concatenate(outs, axis=0)[:cfg.N]
    return np.ascontiguousarray(full.astype(np.float32)), res


def kernel(h, W, attn_l, attn_r, bias, src, dst):
    cfg = Cfg()
    out, _ = _run(cfg, dict(h=h, W=W, attn_l=attn_l, attn_r=attn_r,
                            bias=bias, src=src, dst=dst))
    return out


def _timed_exec(nc, cfg, in_maps, iters=8):
    """Returns a closure measuring pipelined per-call wall time (s)."""
    import time
    import jax
    from jax.experimental.shard_map import shard_map
    from jax.sharding import Mesh, NamedSharding, PartitionSpec
    from concourse import bass2jax

    bass2jax.install_neuronx_cc_hook()
    pname = nc.partition_id_tensor.name if nc.partition_id_tensor else None
    in_names, out_names, out_avals, zero_outs = [], [], [], []
    for alloc in nc.m.functions[0].allocations:
        if not isinstance(alloc, mybir.MemoryLocationSet):
            continue
        name = alloc.memorylocations[0].name
        if alloc.kind == "ExternalInput":
            if name != pname:
                in_names.append(name)
        elif alloc.kind == "ExternalOutput":
            shape = tuple(alloc.tensor_shape)
            dtype = mybir.dt.np(alloc.dtype)
            out_names.append(name)
            out_avals.append(jax.core.ShapedArray(shape, dtype))
            zero_outs.append(np.zeros(shape, dtype))
    n_params = len(in_names)
    all_names = in_names + out_names + ([pname] if pname else [])

    def _body(*args):
        operands = list(args)
        if pname is not None:
            operands.append(bass2jax.partition_id_tensor())
        outs = bass2jax._bass_exec_p.bind(
            *operands,
            out_avals=tuple(out_avals),
            in_names=tuple(all_names),
            out_names=tuple(out_names),
            lowering_input_output_aliases=(),
            sim_require_finite=True,
            sim_require_nnan=True,
            nc=nc,
        )
        return tuple(outs)

    n = cfg.ncores
    devices = jax.devices()[:n]
    mesh = Mesh(np.asarray(devices), ("core",))
    spec = PartitionSpec("core")
    fn = jax.jit(shard_map(_body, mesh=mesh,
                           in_specs=(spec,) * (n_params + len(out_names)),
                           out_specs=(spec,) * len(out_names),
                           check_rep=False),
                 keep_unused=True)
    sh = NamedSharding(mesh, spec)
    args = [
        jax.device_put(
            np.concatenate([np.asarray(in_maps[c][nm]) for c in range(n)],
                           axis=0), sh)
        for nm in in_names
    ] + [
        jax.device_put(np.zeros((n * z.shape[0], *z.shape[1:]), z.dtype), sh)
        for z in zero_outs
    ]
    out = fn(*args)
    jax.block_until_ready(out)

    def timed_batch():
        t0 = time.perf_counter()
        outs = [fn(*args) for _ in range(iters)]
        jax.block_until_ready(outs)
        return (time.perf_counter() - t0) / iters
    return timed_batch


def timed_run(cfg, inputs, iters=8, k=8, ablate=None):
    """Device-time estimate (ns) via repeat-variant difference:
    (t_k - t_1) / (k - 1) cancels host/axon per-call dispatch overhead."""
    idx_dev, dstl_col, dstl_rep, t_regs = preprocess(
        cfg, np.asarray(inputs["src"]), np.asarray(inputs["dst"]))
    in_maps = make_in_maps(cfg, idx_dev, dstl_col, dstl_rep,
                           np.asarray(inputs["h"], np.float32),
                           np.asarray(inputs["W"], np.float32),
                           np.asarray(inputs["attn_l"], np.float32),
                           np.asarray(inputs["attn_r"], np.float32),
                           np.asarray(inputs["bias"], np.float32))
    bz = not np.asarray(inputs["bias"]).any()
    batches = {}
    for rep in (1, k):
        key = (cfg.N, cfg.E, cfg.ncores, cfg.nwin, t_regs, rep, ablate, bz)
        if key not in _CACHE:
            _CACHE[key] = build(cfg, t_regs, repeat=rep, ablate=ablate,
                                bias_zero=bz)
        batches[rep] = _timed_exec(_CACHE[key], cfg, in_maps, iters=iters)
    times = {1: float("inf"), k: float("inf")}
    for _ in range(8):           # interleave to cancel drift
        for rep in (1, k):
            times[rep] = min(times[rep], batches[rep]())
    dt = (times[k] - times[1]) / (k - 1)
    print(f"  t1={times[1]*1e3:.3f} ms  t{k}={times[k]*1e3:.3f} ms")
    return dt * 1e9


# revision 40
# speedup vs baseline: 1.1041x; 1.1041x over previous
"""GAT layer (DGL GATConv + ELU + residual) as a Bass/Tile kernel on 8 TRN2 NeuronCores.

Strategy (edge parallelism, dst-sharded; v2 — fp8 gather table, fused el,
on-chip er expansion):
  - Sort edges by (dst-window, src-region, src) on host; shard contiguous
    dst-node ranges across the 8 cores (6272 nodes/core = 49 windows of 128).
    Each core owns all incoming edges of its node range, so softmax +
    scatter-add are core-local and no collective is needed.
  - Phase A (replicated on every core): one pass of h @ [W | W@Al] produces
    gather-table rows [feat fp8e4m3 (256B) | el fp8 (4B) | pad] in 512B slots,
    written to DRAM ([50176, 512B]).  el = <feat, attn_l> rides along as 4
    extra GEMM columns (W@Al precomputed on host), so no per-edge el compute.
  - Phase A2: er_own = hTo^T @ (W@Ar) for the core's own 6272 nodes, kept
    SBUF-resident ([128, 49*4] bf16).
  - Phase B per 128-node window: one dma_gather per src-region (3 regions of
    <32768 rows each, so int16 indices reach them; slot pads gather row 0 of
    the region and are masked by the one-hots).  Dst-local one-hots are built on DVE
    in BOTH orientations (oh1[edge, node] batched dual-broadcast is_equal,
    ohT[node, edge] from a host-uploaded replicated dst-local int8 row).
    ohT expands er to edges via tiny PE matmuls; oh1 scatter-adds the
    exp-weighted bf16 messages + softmax denominators into a PSUM
    accumulator ([128, 260] f32).  Finalize: /denom, +bias, ELU, +h residual,
    bf16 output (upcast on host).
"""

import sys

for p in ("/opt/trn_rl_repo",):
    if p not in sys.path:
        sys.path.insert(0, p)

import numpy as np

import concourse.bass as bass
import concourse.bacc as bacc
import concourse.mybir as mybir
import concourse.tile as tile
from concourse.bass_utils import run_bass_kernel_spmd

F32 = mybir.dt.float32
BF16 = mybir.dt.bfloat16
FP8 = mybir.dt.float8e4
I8 = mybir.dt.int8
I16 = mybir.dt.int16
AF = mybir.ActivationFunctionType
OP = mybir.AluOpType

P = 128          # partitions / window size
ROWB = 512       # gather-table row bytes (fp8 elems)
ELO = 256        # byte offset of el fp8x4 within a row
ROWP = 260       # payload bytes per row (256 feat + 4 el, all fp8)
RSPLIT = 16768   # src-region width (< 32768 so int16 indices reach all rows)
GMAXT = 8        # max 128-idx tiles per dma_gather (desc-ring capacity 1024)

NP_BF16 = mybir.dt.np(BF16)
NP_FP8 = mybir.dt.np(FP8)


class Cfg:
    def __init__(self, N=50000, E=800000, H=4, D=64, ncores=8, nwin=49,
                 neg_slope=0.2):
        self.N, self.E, self.H, self.D = N, E, H, D
        self.F = H * D
        self.ncores = ncores
        self.nwin = nwin                  # windows (128 nodes) per core
        self.npc = nwin * P               # nodes per core
        self.npad = self.npc * ncores     # padded total nodes
        assert self.npad >= N
        self.neg_slope = neg_slope
        self.nreg = 3


def preprocess(cfg, src, dst):
    """Sort edges by (dst-window, src-region, src); build slot layouts with
    idx=-1 pads (skipped by dma_gather)."""
    s0 = np.asarray(src).astype(np.int64)
    d0 = np.asarray(dst).astype(np.int64)
    gwin = d0 // P
    region = np.minimum(s0 // RSPLIT, cfg.nreg - 1)
    order = np.lexsort((s0, region, gwin))
    s, d, region = s0[order], d0[order], region[order]
    dloc = (d % P).astype(np.int64)
    ngw = cfg.ncores * cfg.nwin
    nreg = cfg.nreg
    cnt_wr = np.zeros((ngw, nreg), np.int64)
    np.add.at(cnt_wr, (gwin[order], region), 1)
    starts = np.concatenate([[0], np.cumsum(cnt_wr.reshape(-1))])
    t_regs = tuple(int(np.ceil(cnt_wr[:, r].max() / P)) for r in range(nreg))
    tpw = sum(t_regs)
    slots = tpw * P
    tbase = np.concatenate([[0], np.cumsum(t_regs)])  # tile offset per region
    idxs = np.zeros((ngw, slots), np.int16)          # pad -> row 0 of region
    dstl = np.full((ngw, slots), 200.0, np.float32)  # pad -> no one-hot match
    dstl8 = np.full((ngw, slots), -1, np.int8)       # pad -> no match vs iota
    for g in range(ngw):
        for r in range(nreg):
            a = starts[g * nreg + r]
            b = starts[g * nreg + r + 1]
            n = b - a
            if n == 0:
                continue
            sb = int(tbase[r]) * P
            idxs[g, sb:sb + n] = s[a:b] - r * RSPLIT
            dstl[g, sb:sb + n] = dloc[a:b]
            dstl8[g, sb:sb + n] = dloc[a:b]
    # dma_gather consumes idx j from [j%16, j//16] (16-row block replicated
    # across the 8 groups of 16 partitions)
    def _dev_layout(a):
        cols = a.reshape(cfg.ncores, cfg.nwin * slots)
        dev = np.zeros((cfg.ncores, P, cfg.nwin * slots // 16), np.int16)
        for c in range(cfg.ncores):
            blk = cols[c].reshape(-1, 16).T          # [16, cols]
            dev[c] = np.tile(blk, (8, 1))
        return dev
    idx_dev = _dev_layout(idxs)
    # dstl (one-hot scalar operand): gather writes slot j -> [j%128, j//128];
    # tile t of window w is dstl_col[:, w*tpw + t]
    dc = dstl.reshape(cfg.ncores, cfg.nwin, tpw, P).transpose(0, 3, 1, 2)
    dstl_col = np.ascontiguousarray(dc.reshape(cfg.ncores, P, cfg.nwin * tpw))
    # dstl_rep (transposed one-hot operand): slot order along the free dim,
    # replicated across all 128 partitions
    dr = dstl8.reshape(cfg.ncores, cfg.nwin * slots)
    dstl_rep = np.ascontiguousarray(
        np.broadcast_to(dr[:, None, :], (cfg.ncores, P, cfg.nwin * slots)))
    return idx_dev, dstl_col, dstl_rep, t_regs


def build(cfg, t_regs, repeat=1, ablate=None, sim_safe=False,
          bias_zero=False):
    """Build the SPMD Bass program. repeat>1 re-emits the computation for
    device-time measurement ((t_k - t_1)/(k-1) cancels dispatch overhead).
    ablate: None | "aonly" | "nogather" | "noscatter" (timing diagnostics).
    sim_safe=True pins all gathers to SWDGE queue 0 (CoreSim cannot model
    multi-queue completion sems; HW counting sems make it safe)."""
    N, F, H, D = cfg.npad, cfg.F, cfg.H, cfg.D
    nwin, npc = cfg.nwin, cfg.npc
    nreg = cfg.nreg
    tpw = sum(t_regs)
    tbase = np.concatenate([[0], np.cumsum(t_regs)]).astype(int)
    KB = F // P            # contraction blocks (2)
    NBC = 1024             # phase-A node chunk
    assert N % NBC == 0
    nchunks = N // NBC
    tpn = NBC // P         # node tiles per chunk (8)
    FE = F + H             # 260 useful GEMM columns (feat + el)
    KW = 7                 # windows per batched finalize group
    assert nwin % KW == 0

    nc = bacc.Bacc("TRN2", target_bir_lowering=False, debug=False,
                   num_devices=cfg.ncores, num_swdge_queues=3)

    hT = nc.dram_tensor("hT", [F, N], BF16, kind="ExternalInput")
    hTo = nc.dram_tensor("hTo", [F, npc], BF16, kind="ExternalInput")
    ho = nc.dram_tensor("ho", [npc, F], BF16, kind="ExternalInput")
    W2 = nc.dram_tensor("W2", [F, FE], BF16, kind="ExternalInput")
    War = nc.dram_tensor("War", [F, H], BF16, kind="ExternalInput")
    brep = nc.dram_tensor("brep", [P, F], F32, kind="ExternalInput")
    idx_d = nc.dram_tensor("idx16", [P, nwin * tpw * P // 16], I16,
                           kind="ExternalInput")
    dstlc_d = nc.dram_tensor("dstlc", [P, nwin * tpw], F32,
                             kind="ExternalInput")
    dstlr_d = nc.dram_tensor("dstlr", [P, nwin * tpw * P], I8,
                             kind="ExternalInput")
    iotaf_d = nc.dram_tensor("iotaf", [P, P], BF16, kind="ExternalInput")
    iotap_d = nc.dram_tensor("iotap", [P, 1], F32, kind="ExternalInput")
    out_d = nc.dram_tensor("out", [npc, F], BF16, kind="ExternalOutput")

    with tile.TileContext(nc) as tc:
        with (
            tc.tile_pool(name="const", bufs=1) as cp,
            tc.tile_pool(name="dram", bufs=1, space="DRAM") as dp,
            tc.tile_pool(name="pa", bufs=3) as pa,
            tc.tile_pool(name="paps", bufs=2, space="PSUM") as paps,
            tc.tile_pool(name="a2ps", bufs=1, space="PSUM") as a2ps,
            tc.tile_pool(name="pg", bufs=4) as pg,
            tc.tile_pool(name="pb", bufs=2) as pb,
            tc.tile_pool(name="pbps", bufs=2, space="PSUM") as pbps,
            tc.tile_pool(name="erps", bufs=2, space="PSUM") as erps,
            tc.tile_pool(name="fin", bufs=2) as fin,
        ):
            # ---------------- constants ----------------
            w_sb = cp.tile([P, KB * FE], BF16)
            for k in range(KB):
                nc.sync.dma_start(w_sb[:, k * FE:(k + 1) * FE],
                                  W2[k * P:(k + 1) * P, :])
            war_sb = cp.tile([P, KB * H], BF16)
            for k in range(KB):
                nc.sync.dma_start(war_sb[:, k * H:(k + 1) * H],
                                  War[k * P:(k + 1) * P, :])
            brep2_sb = cp.tile([P, FE], F32)
            nc.sync.dma_start(brep2_sb[:, 0:F], brep[:])
            nc.vector.memset(brep2_sb[:, F:FE], 0.0)
            idx_sb = cp.tile([P, nwin * tpw * P // 16], I16)
            nc.sync.dma_start(idx_sb[:], idx_d[:])
            dstlc_sb = cp.tile([P, nwin * tpw], F32)
            nc.sync.dma_start(dstlc_sb[:], dstlc_d[:])
            iota_f = cp.tile([P, P], BF16)
            nc.sync.dma_start(iota_f[:], iotaf_d[:])
            iop_f = cp.tile([P, 1], F32)
            nc.sync.dma_start(iop_f[:], iotap_d[:])
            hto_sb = cp.tile([P, KB * npc], BF16)
            for k in range(KB):
                nc.sync.dma_start(hto_sb[:, k * npc:(k + 1) * npc],
                                  hTo[k * P:(k + 1) * P, :])
            er_sb = cp.tile([P, nwin * H], BF16)

            Tfeat = dp.tile([N, ROWB], FP8)
            gidx = [0]   # global gather counter: DMASW lane = gidx%8 is
                         # queue-locked, so queue must be a function of it

            def _emit_phases():
                # ---------------- phase A2: own-range er ----------------
                for w in range(nwin):
                    ps = a2ps.tile([P, H], F32, tag="a2")
                    for k in range(KB):
                        nc.tensor.matmul(
                            ps[:],
                            lhsT=hto_sb[:, k * npc + w * P:k * npc + (w + 1) * P],
                            rhs=war_sb[:, k * H:(k + 1) * H],
                            start=(k == 0), stop=(k == KB - 1))
                    nc.vector.tensor_copy(er_sb[:, w * H:(w + 1) * H], ps[:])

                # ---------------- phase A: gather table ----------------
                for b in range(nchunks):
                    hts = []
                    for k in range(KB):
                        ht = pa.tile([P, NBC], BF16, tag="ht")
                        nc.sync.dma_start(
                            ht[:], hT[k * P:(k + 1) * P, b * NBC:(b + 1) * NBC])
                        hts.append(ht)
                    fo = pa.tile([P, tpn * ROWP], FP8, tag="fo")
                    for i in range(tpn):
                        ps = paps.tile([P, FE], F32, tag="pa")
                        for k in range(KB):
                            nc.tensor.matmul(
                                ps[:], lhsT=hts[k][:, i * P:(i + 1) * P],
                                rhs=w_sb[:, k * FE:(k + 1) * FE],
                                start=(k == 0), stop=(k == KB - 1))
                        nc.vector.tensor_tensor(
                            out=fo[:, i * ROWP:(i + 1) * ROWP], in0=ps[:],
                            in1=brep2_sb[:], op=OP.add)
                    nc.sync.dma_start(
                        Tfeat[b * NBC:(b + 1) * NBC, 0:ROWP].rearrange(
                            "(i p) f -> p i f", p=P),
                        fo[:].rearrange("p (i f) -> p i f", f=ROWP))

                # ---------------- phase B: edges ----------------
                if ablate == "aonly":
                    for w in range(nwin):
                        how = fin.tile([P, F], BF16, tag="how")
                        nc.sync.dma_start(how[:], ho[w * P:(w + 1) * P, :])
                        nc.sync.dma_start(out_d[w * P:(w + 1) * P, :], how[:])
                    return
                for w in range(nwin):
                    base = w * tpw
                    icol = base * P // 16      # idx col base for this window
                    G = pg.tile([P, tpw * ROWB], FP8, tag="G")
                    if ablate == "nogather":
                        nc.vector.memset(G[:, 0:8], 0.0)
                    else:
                        for r in range(nreg):
                            for c0 in range(0, t_regs[r], GMAXT):
                                ct = min(GMAXT, t_regs[r] - c0)
                                rt0 = int(tbase[r]) + c0
                                nc.gpsimd.dma_gather(
                                    out_ap=G[:, rt0 * ROWB:(rt0 + ct) * ROWB]
                                        .rearrange("p (t f) -> p t f", f=ROWB),
                                    in_ap=Tfeat[r * RSPLIT:
                                                min((r + 1) * RSPLIT, N), :],
                                    idxs_ap=idx_sb[:, icol + rt0 * 8:
                                                   icol + (rt0 + ct) * 8],
                                    num_idxs=ct * P, num_idxs_reg=ct * P,
                                    elem_size=ROWB, single_packet=False,
                                    queue_num=0 if sim_safe
                                    else (gidx[0] % 8) % 3)
                                gidx[0] += 1
                    g3 = G[:].rearrange("p (t f) -> p t f", f=ROWB)

                    # one-hots in both orientations
                    dr = pb.tile([P, tpw * P], I8, tag="dr")
                    nc.sync.dma_start(dr[:], dstlr_d[:, base * P:(base + tpw) * P])
                    oh1 = pb.tile([P, tpw * P], BF16, tag="oh1")
                    nc.vector.tensor_tensor(
                        out=oh1[:].rearrange("p (t j) -> p t j", j=P),
                        in0=iota_f[:].unsqueeze(1).to_broadcast([P, tpw, P]),
                        in1=dstlc_sb[:, base:base + tpw].unsqueeze(2)
                            .to_broadcast([P, tpw, P]),
                        op=OP.is_equal)
                    ohT = pb.tile([P, tpw * P], BF16, tag="ohT")
                    nc.vector.tensor_scalar(
                        out=ohT[:], in0=dr[:], scalar1=iop_f[:],
                        scalar2=None, op0=OP.is_equal)

                    # er[dst] per edge via transposed one-hot matmul
                    erp = erps.tile([P, tpw * H], F32, tag="erp")
                    for t in range(tpw):
                        nc.tensor.matmul(
                            erp[:, t * H:(t + 1) * H],
                            lhsT=ohT[:, t * P:(t + 1) * P],
                            rhs=er_sb[:, w * H:(w + 1) * H],
                            start=True, stop=True)

                    # scores -> leaky relu -> exp
                    sc = pb.tile([P, tpw * H], F32, tag="sc")
                    nc.vector.tensor_tensor(
                        out=sc[:].rearrange("p (t h) -> p t h", h=H),
                        in0=g3[:, :, ELO:ROWP],
                        in1=erp[:].rearrange("p (t h) -> p t h", h=H),
                        op=OP.add)
                    e1 = pb.tile([P, tpw * H], F32, tag="e1")
                    nc.scalar.activation(e1[:], sc[:], AF.Exp)
                    e2 = pb.tile([P, tpw * H], F32, tag="e2")
                    nc.scalar.activation(e2[:], sc[:], AF.Exp,
                                         scale=cfg.neg_slope)
                    ex = pb.tile([P, tpw * H], F32, tag="ex")
                    nc.vector.tensor_tensor(out=ex[:], in0=e1[:], in1=e2[:],
                                            op=OP.max)

                    # msg = feat_src * ex (broadcast over d), ex at col 256;
                    # ex is cast to bf16 once (msgx cols 256:260) and reused as
                    # the mult operand so numerator/denominator stay consistent
                    msgx = pb.tile([P, tpw * FE], BF16, tag="msgx")
                    mx3 = msgx[:].rearrange("p (t x) -> p t x", x=FE)
                    nc.vector.tensor_copy(
                        mx3[:, :, F:FE], ex[:].rearrange("p (t h) -> p t h", h=H))
                    nc.vector.tensor_tensor(
                        out=mx3[:, :, 0:F].rearrange(
                            "p t (h d) -> p t h d", d=D),
                        in0=g3[:, :, 0:F].rearrange(
                            "p t (h d) -> p t h d", d=D),
                        in1=mx3[:, :, F:FE].unsqueeze(3)
                            .to_broadcast([P, tpw, H, D]),
                        op=OP.mult)

                    # scatter-add into node accumulator
                    acc = pbps.tile([P, FE], F32, tag="acc")
                    if ablate == "noscatter":
                        nc.tensor.matmul(acc[:], lhsT=oh1[:, 0:P],
                                         rhs=msgx[:, 0:FE],
                                         start=True, stop=True)
                    else:
                        for t in range(tpw):
                            nc.tensor.matmul(
                                acc[:], lhsT=oh1[:, t * P:(t + 1) * P],
                                rhs=msgx[:, t * FE:(t + 1) * FE],
                                start=(t == 0), stop=(t == tpw - 1))

                    # finalize window
                    how = fin.tile([P, F], BF16, tag="how")
                    nc.sync.dma_start(how[:], ho[w * P:(w + 1) * P, :])
                    den = fin.tile([P, H], F32, tag="den")
                    nc.vector.tensor_scalar_max(den[:], acc[:, F:FE], 1e-30)
                    rden = fin.tile([P, H], F32, tag="rden")
                    nc.vector.reciprocal(rden[:], den[:])
                    rst = fin.tile([P, F], F32, tag="rst")
                    nc.vector.tensor_tensor(
                        out=rst[:].rearrange("p (h d) -> p h d", d=D),
                        in0=acc[:, 0:F].rearrange("p (h d) -> p h d", d=D),
                        in1=rden[:].unsqueeze(2).to_broadcast([P, H, D]),
                        op=OP.mult)
                    # ELU: max(x,0)-1 + exp(min(x,0)); then + h residual
                    emin = fin.tile([P, F], F32, tag="emin")
                    nc.vector.tensor_scalar_min(emin[:], rst[:], 0.0)
                    eexp = fin.tile([P, F], F32, tag="eexp")
                    nc.scalar.activation(eexp[:], emin[:], AF.Exp)
                    nc.vector.tensor_scalar(out=rst[:], in0=rst[:],
                                            scalar1=0.0, scalar2=-1.0,
                                            op0=OP.max, op1=OP.add)
                    ot = fin.tile([P, F], BF16, tag="ot")
                    nc.vector.tensor_tensor(out=ot[:], in0=rst[:],
                                            in1=eexp[:], op=OP.add)
                    nc.vector.tensor_tensor(out=ot[:], in0=ot[:], in1=how[:],
                                            op=OP.add)
                    nc.sync.dma_start(out_d[w * P:(w + 1) * P, :], ot[:])

            for _rep in range(repeat):
                _emit_phases()

    nc.compile()
    return nc


def make_in_maps(cfg, idx_dev, dstl_col, dstl_rep, h, W, attn_l, attn_r, bias):
    F, H, D = cfg.F, cfg.H, cfg.D
    h = np.asarray(h, np.float64)
    W64 = np.asarray(W, np.float64)
    Al = np.zeros((F, H))
    Ar = np.zeros((F, H))
    al = np.asarray(attn_l, np.float64)
    ar = np.asarray(attn_r, np.float64)
    for hh in range(H):
        Al[hh * D:(hh + 1) * D, hh] = al[hh]
        Ar[hh * D:(hh + 1) * D, hh] = ar[hh]
    W2 = np.concatenate([W64, W64 @ Al], axis=1).astype(NP_BF16)   # [F, 260]
    War = (W64 @ Ar).astype(NP_BF16)                               # [F, 4]

    h_pad = np.zeros((cfg.npad, F), np.float64)
    h_pad[:cfg.N] = h
    hT = np.ascontiguousarray(h_pad.T).astype(NP_BF16)
    brep = np.tile(np.asarray(bias, np.float32).reshape(1, F), (P, 1))
    iotaf = np.tile(np.arange(P, dtype=np.float32)[None, :],
                    (P, 1)).astype(NP_BF16)
    iotap = np.arange(P, dtype=np.float32)[:, None].copy()
    in_maps = []
    for c in range(cfg.ncores):
        lo, hi = c * cfg.npc, (c + 1) * cfg.npc
        in_maps.append({
            "hT": hT,
            "hTo": np.ascontiguousarray(hT[:, lo:hi]),
            "ho": h_pad[lo:hi].astype(NP_BF16),
            "W2": W2,
            "War": War,
            "brep": brep,
            "idx16": idx_dev[c],
            "dstlc": dstl_col[c],
            "dstlr": dstl_rep[c],
            "iotaf": iotaf,
            "iotap": iotap,
        })
    return in_maps


_CACHE = {}


def _run(cfg, inputs, **spmd_kwargs):
    h = np.asarray(inputs["h"], np.float32)
    W = np.asarray(inputs["W"], np.float32)
    attn_l = np.asarray(inputs["attn_l"], np.float32)
    attn_r = np.asarray(inputs["attn_r"], np.float32)
    bias = np.asarray(inputs["bias"], np.float32)
    src = np.asarray(inputs["src"])
    dst = np.asarray(inputs["dst"])

    idx_dev, dstl_col, dstl_rep, t_regs = preprocess(cfg, src, dst)
    bz = not np.asarray(inputs["bias"]).any()
    key = (cfg.N, cfg.E, cfg.ncores, cfg.nwin, t_regs, bz)
    if key not in _CACHE:
        _CACHE[key] = build(cfg, t_regs, bias_zero=bz)
    nc = _CACHE[key]
    in_maps = make_in_maps(cfg, idx_dev, dstl_col, dstl_rep, h, W,
                           attn_l, attn_r, bias)
    res = run_bass_kernel_spmd(nc, in_maps, list(range(cfg.ncores)),
                               **spmd_kwargs)
    outs = [res.results[c]["out"] for c in range(cfg.ncores)]
    full = np.concatenate(outs, axis=0)[:cfg.N]
    return np.ascontiguousarray(full.astype(np.float32)), res


def kernel(h, W, attn_l, attn_r, bias, src, dst):
    cfg = Cfg()
    out, _ = _run(cfg, dict(h=h, W=W, attn_l=attn_l, attn_r=attn_r,
                            bias=bias, src=src, dst=dst))
    return out


def _timed_exec(nc, cfg, in_maps, iters=8):
    """Returns a closure measuring pipelined per-call wall time (s)."""
    import time
    import jax
    from jax.experimental.shard_map import shard_map
    from jax.sharding import Mesh, NamedSharding, PartitionSpec
    from concourse import bass2jax

    bass2jax.install_neuronx_cc_hook()
    pname = nc.partition_id_tensor.name if nc.partition_id_tensor else None
    in_names, out_names, out_avals, zero_outs = [], [], [], []
    for alloc in nc.m.functions[0].allocations:
        if not isinstance(alloc, mybir.MemoryLocationSet):
            continue
        name = alloc.memorylocations[0].name
        if alloc.kind == "ExternalInput":
            if name != pname:
                in_names.append(name)
        elif alloc.kind == "ExternalOutput":
            shape = tuple(alloc.tensor_shape)
            dtype = mybir.dt.np(alloc.dtype)
            out_names.append(name)
            out_avals.append(jax.core.ShapedArray(shape, dtype))
            zero_outs.append(np.zeros(shape, dtype))
    n_params = len(in_names)
    all_names = in_names + out_names + ([pname] if pname else [])

    def _body(*args):
        operands = list(args)
        if pname is not None:
            operands.append(bass2jax.partition_id_tensor())
        outs = bass2jax._bass_exec_p.bind(
            *operands,
            out_avals=tuple(out_avals),
            in_names=tuple(all_names),
            out_names=tuple(out_names),
            lowering_input_output_aliases=(),
            sim_require_finite=True,
            sim_require_nnan=True,
            nc=nc,
        )
        return tuple(outs)

    n = cfg.ncores
    devices = jax.devices()[:n]
    mesh = Mesh(np.asarray(devices), ("core",))
    spec = PartitionSpec("core")
    fn = jax.jit(shard_map(_body, mesh=mesh,
                           in_specs=(spec,) * (n_params + len(out_names)),
                           out_specs=(spec,) * len(out_names),
                           check_rep=False),
                 keep_unused=True)
    sh = NamedSharding(mesh, spec)
    args = [
        jax.device_put(
            np.concatenate([np.asarray(in_maps[c][nm]) for c in range(n)],
                           axis=0), sh)
        for nm in in_names
    ] + [
        jax.device_put(np.zeros((n * z.shape[0], *z.shape[1:]), z.dtype), sh)
        for z in zero_outs
    ]
    out = fn(*args)
    jax.block_until_ready(out)

    def timed_batch():
        t0 = time.perf_counter()
        outs = [fn(*args) for _ in range(iters)]
        jax.block_until_ready(outs)
        return (time.perf_counter() - t0) / iters
    return timed_batch


def timed_run(cfg, inputs, iters=8, k=8, ablate=None):
    """Device-time estimate (ns) via repeat-variant difference:
    (t_k - t_1) / (k - 1) cancels host/axon per-call dispatch overhead."""
    idx_dev, dstl_col, dstl_rep, t_regs = preprocess(
        cfg, np.asarray(inputs["src"]), np.asarray(inputs["dst"]))
    in_maps = make_in_maps(cfg, idx_dev, dstl_col, dstl_rep,
                           np.asarray(inputs["h"], np.float32),
                           np.asarray(inputs["W"], np.float32),
                           np.asarray(inputs["attn_l"], np.float32),
                           np.asarray(inputs["attn_r"], np.float32),
                           np.asarray(inputs["bias"], np.float32))
    bz = not np.asarray(inputs["bias"]).any()
    batches = {}
    for rep in (1, k):
        key = (cfg.N, cfg.E, cfg.ncores, cfg.nwin, t_regs, rep, ablate, bz)
        if key not in _CACHE:
            _CACHE[key] = build(cfg, t_regs, repeat=rep, ablate=ablate,
                                bias_zero=bz)
        batches[rep] = _timed_exec(_CACHE[key], cfg, in_maps, iters=iters)
    times = {1: float("inf"), k: float("inf")}
    for _ in range(8):           # interleave to cancel drift
        for rep in (1, k):
            times[rep] = min(times[rep], batches[rep]())
    dt = (times[k] - times[1]) / (k - 1)
    print(f"  t1={times[1]*1e3:.3f} ms  t{k}={times[k]*1e3:.3f} ms")
    return dt * 1e9
